# revision 39
# baseline (speedup 1.0000x reference)
"""DenseTopKSAE kernel for Trainium2 (8 NeuronCores, Bass/Tile).

Sharding: expert-parallel over R -- core r owns SAE r (encoder_w[r],
decoder_w[r], x[:, r, :]) and produces out[:, r, :]. No collectives.

Host prep (inside kernel(), numpy): per core r, everything the device
would otherwise spend PE/DVE/scalar cycles on -- transposes and fp16
hi/lo splitting -- is done up front:
  - xcT   = hi/lo fp16 split of (x[:,r,:] - decoder_b[r]).T   [2, C, B]
  - ewT   = hi/lo fp16 split of 64 * encoder_w[r].T           [2, C, D]
  - eb16  = hi/lo fp16 split of 64 * encoder_b[r]             [2, D]
  - dwT16 = (decoder_w[r].T / 64) cast fp16                   [D, C]
  - db16  = hi/lo fp16 split of decoder_b[r]                  [2, C]
The 64x scale keeps the lo split terms out of fp16-subnormal range; h
and the threshold carry the 64x scale, which cancels in the decode
matmul against the 1/64-scaled decoder weights.

Per-core pipeline:
  1. encode  h64 = xcT.T @ ewT (+ 64*eb), 3-term hi/lo split
     (xh*wh + xh*wl + xl*wh). The main term runs in fp16 (1 cyc/row);
     the two correction terms run as fp8(e4m3) DoubleRow matmuls at
     0.5 cyc/row, pre-scaled by 2^11 on host so the lo splits sit in
     fp8 range, and combined with the main term + eb during the DVE
     PSUM drain. Encode h rel err ~1e-5 -> end-to-end 6.3e-3; the
     precision matters because top-k swaps near the threshold cost
     ~0.23 rel err per affected row (a 1-pass float32r encode measures
     2.1e-2 > the 2e-2 gate; pure fp16x3 gives 2.8e-4 but is ~25%
     slower). h staged fp32 to DRAM; top-8 candidates per 256-chunk
     collected on DVE along the way.
  2. top-k threshold: rounds of max8+match_replace on the candidates
     give the k-th largest per row (valid while no 256-chunk holds >8
     of a row's top-k; worst observed = 6 on this data). Run per
     b-tile inside the last encode slab so decode isn't gated on a
     serial threshold pass.
  3. decode: hm = (h >= t) * h fused on DVE (exact top-k incl. relu
     since t>0), cast fp16, PE-transpose; out = hmT.T @ dwT16 fp16
     matmuls, db added via K=1 ones-matmuls on the first d-block,
     fp32 accum in SBUF, per-b-tile output DMA as soon as the last
     d-block lands.
"""

import ml_dtypes
import numpy as np

import concourse.bass as bass
import concourse.mybir as mybir
import concourse.tile as tile
from concourse import bacc
from concourse.bass_utils import run_bass_kernel_spmd

F32 = mybir.dt.float32
F32R = mybir.dt.float32r
F16 = mybir.dt.float16
AF = mybir.ActivationFunctionType
ALU = mybir.AluOpType
P = 128
NEG = -3.0e38

# problem dims (hardcoded per spec; asserted at runtime)
B, R, C, D = 1024, 8, 1024, 16384
N_CORES = 8

SLAB = 1024     # encode/decode d-slab (DMA granularity, 4KB lines)
MMW = 512       # PSUM matmul tile width
CHUNK = 256     # candidate chunk (top-8 per chunk must cover top-k)

# "fp32r": single-pass encode, PE truncates operands to e10m11 (h err
#   ~1.5e-4 -> end-to-end ~1.9e-2, thin margin vs the 2e-2 gate).
# "fp16x3": 3-term hi/lo fp16 split (h err ~1e-6, end-to-end 2.8e-4).
#   Weights/x pre-split on host at 64x scale (keeps lo terms normal);
#   h/threshold carry the 64x scale, decoder weights pre-scaled 1/64.
# "fp8dr": T1 = xh*wh in fp16 + correction terms xh*wl and xl*wh as
#   fp8(e4m3) DoubleRow matmuls at 0.5 cyc/row (both at a common 2^11
#   scale, combined with T1 in the DVE drain). Encode h err ~2e-5 ->
#   end-to-end ~1e-2 est, ~2x margin vs the 2e-2 gate.
ENCODE_MODE = "fp8dr"
WSCALE = 64.0
F8SCALE = 2048.0   # 2^11: lifts the lo split terms into fp8 range
F8 = mybir.dt.float8e4
DR = mybir.MatmulPerfMode.DoubleRow
# "pe": is_transpose matmuls + scalar copy for the masked-h transpose.
# ("dma" = XBAR dma_start_transpose: correct but measured 340us SLOWER —
# the DMA/queue cost far exceeds the ~90us of PE it frees.)
TRANSPOSE_VIA = "pe"


def _mk_identity(nc, ident, fill):
    nc.gpsimd.memset(ident, 0.0)
    nc.gpsimd.affine_select(
        out=ident, in_=ident, compare_op=ALU.not_equal, fill=fill,
        base=0, pattern=[[-1, ident.shape[0]]], channel_multiplier=1,
    )


def _phase1_encode(nc, tc, ewT_d, eb16_d, h_d, xcT, xcT_d, cand, ones16,
                   nb, nct, nslab, t_sb, k):
    """h = xcT.T @ ewT + eb -> DRAM; top-8 candidates per CHUNK.

    On the last slab, each b-tile's threshold is computed right after its
    final candidate write so the decode phase isn't gated on a serial
    threshold pass."""
    with (
        tc.tile_pool(name="encw", bufs=2) as encw,
        tc.tile_pool(name="ench", bufs=3) as ench,
        tc.tile_pool(name="ph2", bufs=2) as ph2,
        tc.tile_pool(name="encps", bufs=4, space="PSUM") as encps,
        tc.tile_pool(name="ebps", bufs=2, space="PSUM") as ebps,
    ):
        for slab in range(nslab):
            d0 = slab * SLAB
            # eb slab first: it feeds the slab's first PE op (the ones-
            # matmul broadcast), so it must not queue behind the big
            # weight-chunk DMAs.
            ebs = encw.tile([1, 2, SLAB], F16, tag="ebs")
            nc.sync.dma_start(
                out=ebs,
                in_=eb16_d[:, d0:d0 + SLAB].rearrange("(o a) d -> o a d", o=1))
            if ENCODE_MODE == "fp32r":
                ew = encw.tile([P, nct, SLAB], F32R, tag="ew")
                for ct in range(nct):
                    nc.sync.dma_start(
                        out=ew[:, ct, :],
                        in_=ewT_d[ct * P:(ct + 1) * P, d0:d0 + SLAB])
                    if slab == 0:
                        nc.sync.dma_start(
                            out=xcT[:, ct, :],
                            in_=xcT_d[ct * P:(ct + 1) * P, :])
            elif ENCODE_MODE == "fp8dr":
                ewh_d, w8h_d, w8l_d = ewT_d
                ew = encw.tile([P, nct, SLAB], F16, tag="ew")
                for ct in range(nct):
                    nc.sync.dma_start(
                        out=ew[:, ct, :],
                        in_=ewh_d[ct * P:(ct + 1) * P, d0:d0 + SLAB])
                w8h = encw.tile([P, 2, nct // 2, SLAB], F8, tag="w8h")
                nc.sync.dma_start(
                    out=w8h,
                    in_=w8h_d[:, :, :, d0:d0 + SLAB].rearrange(
                        "j p i d -> p i j d"))
                w8l = encw.tile([P, 2, nct // 2, SLAB], F8, tag="w8l")
                nc.sync.dma_start(
                    out=w8l,
                    in_=w8l_d[:, :, :, d0:d0 + SLAB].rearrange(
                        "j p i d -> p i j d"))
                if slab == 0:
                    xh16, x8h, x8l = xcT["h16"], xcT["x8h"], xcT["x8l"]
                    xh16_d, x8h_d, x8l_d = xcT_d
                    for ct in range(nct):
                        nc.sync.dma_start(
                            out=xh16[:, ct, :],
                            in_=xh16_d[ct * P:(ct + 1) * P, :])
                    nc.sync.dma_start(
                        out=x8h, in_=x8h_d.rearrange("j p i b -> p i j b"))
                    nc.sync.dma_start(
                        out=x8l, in_=x8l_d.rearrange("j p i b -> p i j b"))
            else:
                ew = encw.tile([P, 2, nct, SLAB], F16, tag="ew")
                for ct in range(nct):
                    nc.sync.dma_start(
                        out=ew[:, :, ct, :],
                        in_=ewT_d[:, ct * P:(ct + 1) * P,
                                  d0:d0 + SLAB].rearrange(
                                      "s p d -> p s d"))
                    if slab == 0:
                        nc.sync.dma_start(
                            out=xcT[:, :, ct, :],
                            in_=xcT_d[:, ct * P:(ct + 1) * P,
                                      :].rearrange("s p b -> p s b"))
            pe_b = ebps.tile([P, SLAB], F32, tag="ebps")
            for h0 in range(0, SLAB, MMW):
                hs = slice(h0, h0 + MMW)
                nc.tensor.matmul(pe_b[:, hs], ones16, ebs[:, 0, hs],
                                 start=True, stop=False)
                nc.tensor.matmul(pe_b[:, hs], ones16, ebs[:, 1, hs],
                                 start=False, stop=True)
            eb_bc = encw.tile([P, SLAB], F32, tag="ebbc")
            nc.scalar.activation(eb_bc, pe_b, AF.Copy)
            for bt in range(nb):
                bsl = slice(bt * P, (bt + 1) * P)
                hsb = ench.tile([P, SLAB], F32, tag="hsb")
                for half in range(SLAB // MMW):
                    h0 = half * MMW
                    hs = slice(h0, h0 + MMW)
                    ph = encps.tile([P, MMW], F32, tag="hps")
                    if ENCODE_MODE == "fp32r":
                        for ct in range(nct):
                            nc.tensor.matmul(ph, xcT[:, ct, bsl],
                                             ew[:, ct, hs],
                                             start=(ct == 0),
                                             stop=(ct == nct - 1))
                    elif ENCODE_MODE == "fp8dr":
                        # T1 operand xh is host-pre-scaled by 2^11 so all
                        # three terms accumulate in ONE PSUM group at the
                        # same scale as the fp8 DoubleRow corrections
                        # (256-deep contraction per DR matmul)
                        for ct in range(nct):
                            nc.tensor.matmul(ph, xcT["h16"][:, ct, bsl],
                                             ew[:, ct, hs],
                                             start=(ct == 0), stop=False)
                        for j in range(nct // 2):
                            nc.tensor.matmul(ph, xcT["x8h"][:, :, j, bsl],
                                             w8l[:, :, j, hs], perf_mode=DR,
                                             start=False, stop=False)
                        for j in range(nct // 2):
                            nc.tensor.matmul(ph, xcT["x8l"][:, :, j, bsl],
                                             w8h[:, :, j, hs], perf_mode=DR,
                                             start=False,
                                             stop=(j == nct // 2 - 1))
                        # single-pass drain: hsb = ph/2^11 + eb
                        nc.vector.scalar_tensor_tensor(
                            out=hsb[:, hs], in0=ph,
                            scalar=float(1.0 / F8SCALE), in1=eb_bc[:, hs],
                            op0=ALU.mult, op1=ALU.add)
                        continue
                    else:
                        for ct in range(nct):
                            nc.tensor.matmul(ph, xcT[:, 0, ct, bsl],
                                             ew[:, 0, ct, hs],
                                             start=(ct == 0), stop=False)
                            nc.tensor.matmul(ph, xcT[:, 0, ct, bsl],
                                             ew[:, 1, ct, hs],
                                             start=False, stop=False)
                            nc.tensor.matmul(ph, xcT[:, 1, ct, bsl],
                                             ew[:, 0, ct, hs],
                                             start=False,
                                             stop=(ct == nct - 1))
                    # drain + eb add in one DVE pass
                    nc.vector.tensor_add(hsb[:, hs], ph, eb_bc[:, hs])
                nc.sync.dma_start(out=h_d[bsl, d0:d0 + SLAB], in_=hsb)
                for ch in range(SLAB // CHUNK):
                    ci = (d0 // CHUNK) + ch
                    nc.vector.max(out=cand[bt][:, ci * 8:(ci + 1) * 8],
                                  in_=hsb[:, ch * CHUNK:(ch + 1) * CHUNK])
                if slab == nslab - 1:
                    # threshold for this b-tile (candidates now complete)
                    rounds = (k + 7) // 8
                    scr = ph2.tile([P, 8], F32, tag="scr")
                    for rnd in range(rounds):
                        nc.vector.max(out=scr, in_=cand[bt])
                        if rnd < rounds - 1:
                            nc.vector.match_replace(
                                out=cand[bt], in_to_replace=scr,
                                in_values=cand[bt], imm_value=NEG)
                    pos = (k - 1) % 8
                    nc.vector.tensor_scalar_max(
                        t_sb[:, bt:bt + 1], scr[:, pos:pos + 1], 1e-30)


def _phase3_decode(nc, tc, dwT_d, h_d, t_sb, db16, ones16, ident16,
                   out_acc, out_d, nb, nct, nslab, b, c):
    ndt = SLAB // P
    ncb = c // MMW
    with (
        tc.tile_pool(name="dech", bufs=3) as dech,
        tc.tile_pool(name="dechm", bufs=2) as dechm,
        tc.tile_pool(name="decw", bufs=2) as decw,
        tc.tile_pool(name="decps", bufs=4, space="PSUM") as decps,
        tc.tile_pool(name="trps", bufs=2, space="PSUM") as trps,
    ):
        def fetch_dwT(d2):
            d0 = d2 * SLAB
            dwT = decw.tile([P, ndt, c], F16, tag="dwT")
            nc.sync.dma_start(
                out=dwT,
                in_=dwT_d[d0:d0 + SLAB, :].rearrange("(a p) c -> p a c", p=P))
            return dwT

        def build_hmT(d2):
            d0 = d2 * SLAB
            hmT = dechm.tile([P, ndt, b], F16, tag="hmT")
            for bt in range(nb):
                bsl = slice(bt * P, (bt + 1) * P)
                hblk = dech.tile([P, SLAB], F32, tag="hldb")
                nc.sync.dma_start(out=hblk, in_=h_d[bsl, d0:d0 + SLAB])
                hm16 = dech.tile([P, SLAB], F16, tag="hm16")
                # hm = (h >= t) * h in one DVE pass
                nc.vector.scalar_tensor_tensor(
                    out=hm16, in0=hblk, scalar=t_sb[:, bt:bt + 1],
                    in1=hblk, op0=ALU.is_ge, op1=ALU.mult)
                if TRANSPOSE_VIA == "dma":
                    nc.sync.dma_start_transpose(hmT[:, :, bsl], hm16)
                else:
                    pw = trps.tile([P, SLAB], F16, tag="hmtr")
                    for dt in range(ndt):
                        nc.tensor.transpose(pw[:, dt * P:(dt + 1) * P],
                                            hm16[:, dt * P:(dt + 1) * P],
                                            ident16)
                    nc.scalar.activation(
                        hmT[:, :, bsl],
                        pw.rearrange("p (a q) -> p a q", q=P), AF.Copy)
            return hmT

        dwT = fetch_dwT(0)
        hmT = build_hmT(0)
        for d2 in range(nslab):
            d0 = d2 * SLAB
            # prefetch + prebuild next slab so the PE never waits on the
            # DVE mask / transpose chain between slabs
            if d2 + 1 < nslab:
                dwT_next = fetch_dwT(d2 + 1)
                hmT_next = build_hmT(d2 + 1)
            for bt in range(nb):
                bsl = slice(bt * P, (bt + 1) * P)
                for cb in range(ncb):
                    cs = slice(cb * MMW, (cb + 1) * MMW)
                    po = decps.tile([P, MMW], F32, tag="ops")
                    first = (d2 == 0)
                    if first:
                        nc.tensor.matmul(po, ones16, db16[:, 0, cs],
                                         start=True, stop=False)
                        nc.tensor.matmul(po, ones16, db16[:, 1, cs],
                                         start=False, stop=False)
                    for dt in range(ndt):
                        nc.tensor.matmul(
                            po, hmT[:, dt, bsl], dwT[:, dt, cs],
                            start=(dt == 0 and not first),
                            stop=(dt == ndt - 1))
                    if first:
                        nc.scalar.activation(out_acc[bt][:, cs], po, AF.Copy)
                    else:
                        nc.vector.tensor_add(out_acc[bt][:, cs],
                                             out_acc[bt][:, cs], po)
                if d2 == nslab - 1:
                    nc.sync.dma_start(out=out_d[bsl, :], in_=out_acc[bt])
            if d2 + 1 < nslab:
                dwT, hmT = dwT_next, hmT_next


def build(k, b=B, c=C, d=D):
    """Build the single-core SPMD program (same program, per-core data)."""
    nb, nct, nslab = b // P, c // P, d // SLAB

    nc = bacc.Bacc("TRN2", target_bir_lowering=False, debug=False,
                   num_devices=N_CORES)
    if ENCODE_MODE == "fp32r":
        xcT_d = nc.declare_dram_parameter("xcT", [c, b], F32R, isOutput=False)
        ewT_d = nc.declare_dram_parameter("ewT", [c, d], F32R, isOutput=False)
    elif ENCODE_MODE == "fp8dr":
        nct2 = c // (2 * P)
        xcT_d = (
            nc.declare_dram_parameter("xcTh", [c, b], F16, isOutput=False),
            nc.declare_dram_parameter("x8h", [nct2, P, 2, b], F8,
                                      isOutput=False),
            nc.declare_dram_parameter("x8l", [nct2, P, 2, b], F8,
                                      isOutput=False),
        )
        ewT_d = (
            nc.declare_dram_parameter("ewTh", [c, d], F16, isOutput=False),
            nc.declare_dram_parameter("w8h", [nct2, P, 2, d], F8,
                                      isOutput=False),
            nc.declare_dram_parameter("w8l", [nct2, P, 2, d], F8,
                                      isOutput=False),
        )
    else:
        xcT_d = nc.declare_dram_parameter("xcT", [2, c, b], F16,
                                          isOutput=False)
        ewT_d = nc.declare_dram_parameter("ewT", [2, c, d], F16,
                                          isOutput=False)
    eb16_d = nc.declare_dram_parameter("eb16", [2, d], F16, isOutput=False)
    dwT_d = nc.declare_dram_parameter("dwT16", [d, c], F16, isOutput=False)
    db16_d = nc.declare_dram_parameter("db16", [2, c], F16, isOutput=False)
    out_d = nc.declare_dram_parameter("out", [b, c], F32, isOutput=True)
    h_d = nc.dram_tensor("h_scratch", [b, d], F32)

    with tile.TileContext(nc) as tc:
        with tc.tile_pool(name="persist", bufs=1) as pp:
            ident16 = pp.tile([P, P], F16, tag="ident16")
            _mk_identity(nc, ident16, 1.0)
            ones16 = pp.tile([1, P], F16, tag="ones16")
            nc.vector.memset(ones16, 1.0)
            db16 = pp.tile([1, 2, c], F16, tag="db16")
            nc.sync.dma_start(
                out=db16, in_=db16_d.rearrange("(o a) q -> o a q", o=1))

            # per-row threshold, one column per b-tile
            t_sb = pp.tile([P, nb], F32, tag="tsb")

            with tc.tile_pool(name="candp", bufs=1) as cp:
                cand = [cp.tile([P, (d // CHUNK) * 8], F32, tag=f"cand{bt}",
                                name=f"cand{bt}") for bt in range(nb)]
                with tc.tile_pool(name="xcpool", bufs=1) as xcp:
                    if ENCODE_MODE == "fp32r":
                        xcT = xcp.tile([P, nct, b], F32R, tag="xcT")
                    elif ENCODE_MODE == "fp8dr":
                        xcT = {
                            "h16": xcp.tile([P, nct, b], F16, tag="xh16",
                                            name="xh16"),
                            "x8h": xcp.tile([P, 2, nct // 2, b], F8,
                                            tag="x8h", name="x8h"),
                            "x8l": xcp.tile([P, 2, nct // 2, b], F8,
                                            tag="x8l", name="x8l"),
                        }
                    else:
                        xcT = xcp.tile([P, 2, nct, b], F16, tag="xcT")
                    _phase1_encode(nc, tc, ewT_d, eb16_d, h_d, xcT, xcT_d,
                                   cand, ones16, nb, nct, nslab, t_sb, k)

            out_acc = [pp.tile([P, c], F32, tag=f"oacc{bt}", name=f"oacc{bt}")
                       for bt in range(nb)]
            _phase3_decode(nc, tc, dwT_d, h_d, t_sb, db16, ones16, ident16,
                           out_acc, out_d, nb, nct, nslab, b, c)
    return nc


def _f16_split(a):
    hi = a.astype(np.float16)
    lo = (a - hi.astype(np.float32)).astype(np.float16)
    return np.stack([hi, lo])


def _il8(a):
    """[c, n] f32 -> [c/256, 128, 2, n] e4m3 DoubleRow interleave.

    Pairing: contraction index c = j*256 + i*128 + p maps to [j, p, i]."""
    c, n = a.shape
    v = a.reshape(c // 256, 2, P, n).transpose(0, 2, 1, 3)
    return np.ascontiguousarray(v).astype(ml_dtypes.float8_e4m3)


def run(x, encoder_w, encoder_b, decoder_w, decoder_b, k, trace=False):
    x = np.asarray(x, dtype=np.float32)
    encoder_w = np.asarray(encoder_w, dtype=np.float32)
    encoder_b = np.asarray(encoder_b, dtype=np.float32)
    decoder_w = np.asarray(decoder_w, dtype=np.float32)
    decoder_b = np.asarray(decoder_b, dtype=np.float32)
    k = int(k)
    b, r, c = x.shape
    d = encoder_w.shape[1]
    assert (b, r, c, d) == (B, R, C, D), (b, r, c, d)

    nc = build(k)
    if not nc.is_finalized():
        nc.finalize()
    in_maps = []
    for i in range(r):
        xc = x[:, i, :] - decoder_b[i][None, :]
        if ENCODE_MODE == "fp32r":
            in_maps.append({
                "xcT": np.ascontiguousarray(xc.T),
                "ewT": np.ascontiguousarray(encoder_w[i].T),
                "eb16": _f16_split(encoder_b[i]),
                "dwT16": np.ascontiguousarray(
                    decoder_w[i].T).astype(np.float16),
                "db16": _f16_split(decoder_b[i]),
            })
        elif ENCODE_MODE == "fp8dr":
            xcT = np.ascontiguousarray(xc.T)
            xh = xcT.astype(np.float16)
            xl = xcT - xh.astype(np.float32)
            ew64 = np.ascontiguousarray(encoder_w[i].T) * np.float32(WSCALE)
            wh = ew64.astype(np.float16)
            wl = ew64 - wh.astype(np.float32)
            in_maps.append({
                "xcTh": (xh.astype(np.float32)
                         * np.float32(F8SCALE)).astype(np.float16),
                "x8h": _il8(xh.astype(np.float32)),
                "x8l": _il8(xl * np.float32(F8SCALE)),
                "ewTh": wh,
                "w8h": _il8(wh.astype(np.float32)),
                "w8l": _il8(wl * np.float32(F8SCALE)),
                "eb16": _f16_split(encoder_b[i] * np.float32(WSCALE)),
                "dwT16": (np.ascontiguousarray(decoder_w[i].T)
                          * np.float32(1.0 / WSCALE)).astype(np.float16),
                "db16": _f16_split(decoder_b[i]),
            })
        else:
            # 64x-scaled hi/lo splits; decoder weights pre-scaled 1/64 so
            # the 64x-scaled masked h cancels in the decode matmul.
            in_maps.append({
                "xcT": _f16_split(np.ascontiguousarray(xc.T)),
                "ewT": _f16_split(
                    np.ascontiguousarray(encoder_w[i].T) * np.float32(WSCALE)),
                "eb16": _f16_split(encoder_b[i] * np.float32(WSCALE)),
                "dwT16": (np.ascontiguousarray(decoder_w[i].T)
                          * np.float32(1.0 / WSCALE)).astype(np.float16),
                "db16": _f16_split(decoder_b[i]),
            })
    res = run_bass_kernel_spmd(nc, in_maps, core_ids=list(range(N_CORES)),
                               trace=trace)
    out = np.empty((b, r, c), dtype=np.float32)
    for i in range(r):
        out[:, i, :] = res.results[i]["out"]
    return out, res


def kernel(x, encoder_w, encoder_b, decoder_w, decoder_b, k):
    out, _ = run(x, encoder_w, encoder_b, decoder_w, decoder_b, k)
    return out



# revision 42
# speedup vs baseline: 1.0215x; 1.0215x over previous
"""DenseTopKSAE kernel for Trainium2 (8 NeuronCores, Bass/Tile).

Sharding: expert-parallel over R -- core r owns SAE r (encoder_w[r],
decoder_w[r], x[:, r, :]) and produces out[:, r, :]. No collectives.

Host prep (inside kernel(), numpy): per core r, everything the device
would otherwise spend PE/DVE/scalar cycles on -- transposes and fp16
hi/lo splitting -- is done up front:
  - xcT   = hi/lo fp16 split of (x[:,r,:] - decoder_b[r]).T   [2, C, B]
  - ewT   = hi/lo fp16 split of 64 * encoder_w[r].T           [2, C, D]
  - eb16  = hi/lo fp16 split of 64 * encoder_b[r]             [2, D]
  - dwT16 = (decoder_w[r].T / 64) cast fp16                   [D, C]
  - db16  = hi/lo fp16 split of decoder_b[r]                  [2, C]
The 64x scale keeps the lo split terms out of fp16-subnormal range; h
and the threshold carry the 64x scale, which cancels in the decode
matmul against the 1/64-scaled decoder weights.

Per-core pipeline:
  1. encode  h64 = xcT.T @ ewT (+ 64*eb), 3-term hi/lo split
     (xh*wh + xh*wl + xl*wh). The main term runs in fp16 (1 cyc/row);
     the two correction terms run as fp8(e4m3) DoubleRow matmuls at
     0.5 cyc/row, pre-scaled by 2^11 on host so the lo splits sit in
     fp8 range, and combined with the main term + eb during the DVE
     PSUM drain. Encode h rel err ~1e-5 -> end-to-end 6.3e-3; the
     precision matters because top-k swaps near the threshold cost
     ~0.23 rel err per affected row (a 1-pass float32r encode measures
     2.1e-2 > the 2e-2 gate; pure fp16x3 gives 2.8e-4 but is ~25%
     slower). h staged fp32 to DRAM; top-8 candidates per 256-chunk
     collected on DVE along the way.
  2. top-k threshold: rounds of max8+match_replace on the candidates
     give the k-th largest per row (valid while no 256-chunk holds >8
     of a row's top-k; worst observed = 6 on this data). Run per
     b-tile inside the last encode slab so decode isn't gated on a
     serial threshold pass.
  3. decode: hm = (h >= t) * h fused on DVE (exact top-k incl. relu
     since t>0), cast fp16, PE-transpose; out = hmT.T @ dwT16 fp16
     matmuls, db added via K=1 ones-matmuls on the first d-block,
     fp32 accum in SBUF, per-b-tile output DMA as soon as the last
     d-block lands.
"""

import ml_dtypes
import numpy as np

import concourse.bass as bass
import concourse.mybir as mybir
import concourse.tile as tile
from concourse import bacc
from concourse.bass_utils import run_bass_kernel_spmd

F32 = mybir.dt.float32
F32R = mybir.dt.float32r
F16 = mybir.dt.float16
AF = mybir.ActivationFunctionType
ALU = mybir.AluOpType
P = 128
NEG = -3.0e38

# problem dims (hardcoded per spec; asserted at runtime)
B, R, C, D = 1024, 8, 1024, 16384
N_CORES = 8

SLAB = 1024     # encode/decode d-slab (DMA granularity, 4KB lines)
MMW = 512       # PSUM matmul tile width
CHUNK = 256     # candidate chunk (top-8 per chunk must cover top-k)

# "fp32r": single-pass encode, PE truncates operands to e10m11 (h err
#   ~1.5e-4 -> end-to-end ~1.9e-2, thin margin vs the 2e-2 gate).
# "fp16x3": 3-term hi/lo fp16 split (h err ~1e-6, end-to-end 2.8e-4).
#   Weights/x pre-split on host at 64x scale (keeps lo terms normal);
#   h/threshold carry the 64x scale, decoder weights pre-scaled 1/64.
# "fp8dr": T1 = xh*wh in fp16 + correction terms xh*wl and xl*wh as
#   fp8(e4m3) DoubleRow matmuls at 0.5 cyc/row. xh is host-pre-scaled
#   by 2^11 so all three terms accumulate in one PSUM group at the
#   same scale; one DVE pass drains ph/2^11 + eb. Encode h err
#   ~1e-5 -> end-to-end 6.27e-3 measured, 3.2x margin vs the 2e-2
#   gate (deterministic on this data).
ENCODE_MODE = "fp8dr"
WSCALE = 64.0
F8SCALE = 2048.0   # 2^11: lifts the lo split terms into fp8 range
F8 = mybir.dt.float8e4
DR = mybir.MatmulPerfMode.DoubleRow
# "pe": is_transpose matmuls + scalar copy for the masked-h transpose.
# ("dma" = XBAR dma_start_transpose: correct but measured 340us SLOWER —
# the DMA/queue cost far exceeds the ~90us of PE it frees.)
TRANSPOSE_VIA = "pe"


def _mk_identity(nc, ident, fill):
    nc.gpsimd.memset(ident, 0.0)
    nc.gpsimd.affine_select(
        out=ident, in_=ident, compare_op=ALU.not_equal, fill=fill,
        base=0, pattern=[[-1, ident.shape[0]]], channel_multiplier=1,
    )


def _phase1_encode(nc, tc, ewT_d, eb16_d, h_d, xcT, xcT_d, cand, ones16,
                   nb, nct, nslab, t_sb, k):
    """h = xcT.T @ ewT + eb -> DRAM; top-8 candidates per CHUNK.

    On the last slab, each b-tile's threshold is computed right after its
    final candidate write so the decode phase isn't gated on a serial
    threshold pass."""
    with (
        tc.tile_pool(name="encw", bufs=2) as encw,
        tc.tile_pool(name="ench", bufs=3) as ench,
        tc.tile_pool(name="ph2", bufs=2) as ph2,
        tc.tile_pool(name="encps", bufs=4, space="PSUM") as encps,
        tc.tile_pool(name="ebps", bufs=2, space="PSUM") as ebps,
    ):
        for slab in range(nslab):
            d0 = slab * SLAB
            # eb slab first: it feeds the slab's first PE op (the ones-
            # matmul broadcast), so it must not queue behind the big
            # weight-chunk DMAs.
            ebs = encw.tile([1, 2, SLAB], F16, tag="ebs")
            nc.sync.dma_start(
                out=ebs,
                in_=eb16_d[:, d0:d0 + SLAB].rearrange("(o a) d -> o a d", o=1))
            if ENCODE_MODE == "fp32r":
                ew = encw.tile([P, nct, SLAB], F32R, tag="ew")
                for ct in range(nct):
                    nc.sync.dma_start(
                        out=ew[:, ct, :],
                        in_=ewT_d[ct * P:(ct + 1) * P, d0:d0 + SLAB])
                    if slab == 0:
                        nc.sync.dma_start(
                            out=xcT[:, ct, :],
                            in_=xcT_d[ct * P:(ct + 1) * P, :])
            elif ENCODE_MODE == "fp8dr":
                ewh_d, w8h_d, w8l_d = ewT_d
                ew = encw.tile([P, nct, SLAB], F16, tag="ew")
                for ct in range(nct):
                    nc.sync.dma_start(
                        out=ew[:, ct, :],
                        in_=ewh_d[ct * P:(ct + 1) * P, d0:d0 + SLAB])
                w8h = encw.tile([P, 2, nct // 2, SLAB], F8, tag="w8h")
                nc.sync.dma_start(
                    out=w8h,
                    in_=w8h_d[:, :, :, d0:d0 + SLAB].rearrange(
                        "j p i d -> p i j d"))
                w8l = encw.tile([P, 2, nct // 2, SLAB], F8, tag="w8l")
                nc.sync.dma_start(
                    out=w8l,
                    in_=w8l_d[:, :, :, d0:d0 + SLAB].rearrange(
                        "j p i d -> p i j d"))
                if slab == 0:
                    xh16, x8h, x8l = xcT["h16"], xcT["x8h"], xcT["x8l"]
                    xh16_d, x8h_d, x8l_d = xcT_d
                    for ct in range(nct):
                        nc.sync.dma_start(
                            out=xh16[:, ct, :],
                            in_=xh16_d[ct * P:(ct + 1) * P, :])
                    nc.sync.dma_start(
                        out=x8h, in_=x8h_d.rearrange("j p i b -> p i j b"))
                    nc.sync.dma_start(
                        out=x8l, in_=x8l_d.rearrange("j p i b -> p i j b"))
            else:
                ew = encw.tile([P, 2, nct, SLAB], F16, tag="ew")
                for ct in range(nct):
                    nc.sync.dma_start(
                        out=ew[:, :, ct, :],
                        in_=ewT_d[:, ct * P:(ct + 1) * P,
                                  d0:d0 + SLAB].rearrange(
                                      "s p d -> p s d"))
                    if slab == 0:
                        nc.sync.dma_start(
                            out=xcT[:, :, ct, :],
                            in_=xcT_d[:, ct * P:(ct + 1) * P,
                                      :].rearrange("s p b -> p s b"))
            pe_b = ebps.tile([P, SLAB], F32, tag="ebps")
            for h0 in range(0, SLAB, MMW):
                hs = slice(h0, h0 + MMW)
                nc.tensor.matmul(pe_b[:, hs], ones16, ebs[:, 0, hs],
                                 start=True, stop=False)
                nc.tensor.matmul(pe_b[:, hs], ones16, ebs[:, 1, hs],
                                 start=False, stop=True)
            eb_bc = encw.tile([P, SLAB], F32, tag="ebbc")
            nc.scalar.activation(eb_bc, pe_b, AF.Copy)
            for bt in range(nb):
                bsl = slice(bt * P, (bt + 1) * P)
                hsb = ench.tile([P, SLAB], F32, tag="hsb")
                for half in range(SLAB // MMW):
                    h0 = half * MMW
                    hs = slice(h0, h0 + MMW)
                    ph = encps.tile([P, MMW], F32, tag="hps")
                    if ENCODE_MODE == "fp32r":
                        for ct in range(nct):
                            nc.tensor.matmul(ph, xcT[:, ct, bsl],
                                             ew[:, ct, hs],
                                             start=(ct == 0),
                                             stop=(ct == nct - 1))
                    elif ENCODE_MODE == "fp8dr":
                        # T1 operand xh is host-pre-scaled by 2^11 so all
                        # three terms accumulate in ONE PSUM group at the
                        # same scale as the fp8 DoubleRow corrections
                        # (256-deep contraction per DR matmul)
                        for ct in range(nct):
                            nc.tensor.matmul(ph, xcT["h16"][:, ct, bsl],
                                             ew[:, ct, hs],
                                             start=(ct == 0), stop=False)
                        for j in range(nct // 2):
                            nc.tensor.matmul(ph, xcT["x8h"][:, :, j, bsl],
                                             w8l[:, :, j, hs], perf_mode=DR,
                                             start=False, stop=False)
                        for j in range(nct // 2):
                            nc.tensor.matmul(ph, xcT["x8l"][:, :, j, bsl],
                                             w8h[:, :, j, hs], perf_mode=DR,
                                             start=False,
                                             stop=(j == nct // 2 - 1))
                        # single-pass drain: hsb = ph/2^11 + eb
                        nc.vector.scalar_tensor_tensor(
                            out=hsb[:, hs], in0=ph,
                            scalar=float(1.0 / F8SCALE), in1=eb_bc[:, hs],
                            op0=ALU.mult, op1=ALU.add)
                        continue
                    else:
                        for ct in range(nct):
                            nc.tensor.matmul(ph, xcT[:, 0, ct, bsl],
                                             ew[:, 0, ct, hs],
                                             start=(ct == 0), stop=False)
                            nc.tensor.matmul(ph, xcT[:, 0, ct, bsl],
                                             ew[:, 1, ct, hs],
                                             start=False, stop=False)
                            nc.tensor.matmul(ph, xcT[:, 1, ct, bsl],
                                             ew[:, 0, ct, hs],
                                             start=False,
                                             stop=(ct == nct - 1))
                    # drain + eb add in one DVE pass
                    nc.vector.tensor_add(hsb[:, hs], ph, eb_bc[:, hs])
                nc.sync.dma_start(out=h_d[bsl, d0:d0 + SLAB], in_=hsb)
                for ch in range(SLAB // CHUNK):
                    ci = (d0 // CHUNK) + ch
                    nc.vector.max(out=cand[bt][:, ci * 8:(ci + 1) * 8],
                                  in_=hsb[:, ch * CHUNK:(ch + 1) * CHUNK])
                if slab == nslab - 1:
                    # threshold for this b-tile (candidates now complete)
                    rounds = (k + 7) // 8
                    scr = ph2.tile([P, 8], F32, tag="scr")
                    for rnd in range(rounds):
                        nc.vector.max(out=scr, in_=cand[bt])
                        if rnd < rounds - 1:
                            nc.vector.match_replace(
                                out=cand[bt], in_to_replace=scr,
                                in_values=cand[bt], imm_value=NEG)
                    pos = (k - 1) % 8
                    nc.vector.tensor_scalar_max(
                        t_sb[:, bt:bt + 1], scr[:, pos:pos + 1], 1e-30)


def _phase3_decode(nc, tc, dwT_d, h_d, t_sb, db16, ones16, ident16,
                   out_acc, out_d, nb, nct, nslab, b, c):
    ndt = SLAB // P
    ncb = c // MMW
    with (
        tc.tile_pool(name="dech", bufs=3) as dech,
        tc.tile_pool(name="dechm", bufs=2) as dechm,
        tc.tile_pool(name="decw", bufs=2) as decw,
        tc.tile_pool(name="decps", bufs=4, space="PSUM") as decps,
        tc.tile_pool(name="trps", bufs=2, space="PSUM") as trps,
    ):
        def fetch_dwT(d2):
            d0 = d2 * SLAB
            dwT = decw.tile([P, ndt, c], F16, tag="dwT")
            nc.sync.dma_start(
                out=dwT,
                in_=dwT_d[d0:d0 + SLAB, :].rearrange("(a p) c -> p a c", p=P))
            return dwT

        def build_hmT(d2):
            d0 = d2 * SLAB
            hmT = dechm.tile([P, ndt, b], F16, tag="hmT")
            for bt in range(nb):
                bsl = slice(bt * P, (bt + 1) * P)
                hblk = dech.tile([P, SLAB], F32, tag="hldb")
                nc.sync.dma_start(out=hblk, in_=h_d[bsl, d0:d0 + SLAB])
                hm16 = dech.tile([P, SLAB], F16, tag="hm16")
                # hm = (h >= t) * h in one DVE pass
                nc.vector.scalar_tensor_tensor(
                    out=hm16, in0=hblk, scalar=t_sb[:, bt:bt + 1],
                    in1=hblk, op0=ALU.is_ge, op1=ALU.mult)
                if TRANSPOSE_VIA == "dma":
                    nc.sync.dma_start_transpose(hmT[:, :, bsl], hm16)
                else:
                    pw = trps.tile([P, SLAB], F16, tag="hmtr")
                    for dt in range(ndt):
                        nc.tensor.transpose(pw[:, dt * P:(dt + 1) * P],
                                            hm16[:, dt * P:(dt + 1) * P],
                                            ident16)
                    nc.scalar.activation(
                        hmT[:, :, bsl],
                        pw.rearrange("p (a q) -> p a q", q=P), AF.Copy)
            return hmT

        dwT = fetch_dwT(0)
        hmT = build_hmT(0)
        for d2 in range(nslab):
            d0 = d2 * SLAB
            # prefetch + prebuild next slab so the PE never waits on the
            # DVE mask / transpose chain between slabs
            if d2 + 1 < nslab:
                dwT_next = fetch_dwT(d2 + 1)
                hmT_next = build_hmT(d2 + 1)
            for bt in range(nb):
                bsl = slice(bt * P, (bt + 1) * P)
                for cb in range(ncb):
                    cs = slice(cb * MMW, (cb + 1) * MMW)
                    po = decps.tile([P, MMW], F32, tag="ops")
                    first = (d2 == 0)
                    if first:
                        nc.tensor.matmul(po, ones16, db16[:, 0, cs],
                                         start=True, stop=False)
                        nc.tensor.matmul(po, ones16, db16[:, 1, cs],
                                         start=False, stop=False)
                    for dt in range(ndt):
                        nc.tensor.matmul(
                            po, hmT[:, dt, bsl], dwT[:, dt, cs],
                            start=(dt == 0 and not first),
                            stop=(dt == ndt - 1))
                    if first:
                        nc.scalar.activation(out_acc[bt][:, cs], po, AF.Copy)
                    else:
                        nc.vector.tensor_add(out_acc[bt][:, cs],
                                             out_acc[bt][:, cs], po)
                if d2 == nslab - 1:
                    nc.sync.dma_start(out=out_d[bsl, :], in_=out_acc[bt])
            if d2 + 1 < nslab:
                dwT, hmT = dwT_next, hmT_next


def build(k, b=B, c=C, d=D):
    """Build the single-core SPMD program (same program, per-core data)."""
    nb, nct, nslab = b // P, c // P, d // SLAB

    nc = bacc.Bacc("TRN2", target_bir_lowering=False, debug=False,
                   num_devices=N_CORES)
    if ENCODE_MODE == "fp32r":
        xcT_d = nc.declare_dram_parameter("xcT", [c, b], F32R, isOutput=False)
        ewT_d = nc.declare_dram_parameter("ewT", [c, d], F32R, isOutput=False)
    elif ENCODE_MODE == "fp8dr":
        nct2 = c // (2 * P)
        xcT_d = (
            nc.declare_dram_parameter("xcTh", [c, b], F16, isOutput=False),
            nc.declare_dram_parameter("x8h", [nct2, P, 2, b], F8,
                                      isOutput=False),
            nc.declare_dram_parameter("x8l", [nct2, P, 2, b], F8,
                                      isOutput=False),
        )
        ewT_d = (
            nc.declare_dram_parameter("ewTh", [c, d], F16, isOutput=False),
            nc.declare_dram_parameter("w8h", [nct2, P, 2, d], F8,
                                      isOutput=False),
            nc.declare_dram_parameter("w8l", [nct2, P, 2, d], F8,
                                      isOutput=False),
        )
    else:
        xcT_d = nc.declare_dram_parameter("xcT", [2, c, b], F16,
                                          isOutput=False)
        ewT_d = nc.declare_dram_parameter("ewT", [2, c, d], F16,
                                          isOutput=False)
    eb16_d = nc.declare_dram_parameter("eb16", [2, d], F16, isOutput=False)
    dwT_d = nc.declare_dram_parameter("dwT16", [d, c], F16, isOutput=False)
    db16_d = nc.declare_dram_parameter("db16", [2, c], F16, isOutput=False)
    out_d = nc.declare_dram_parameter("out", [b, c], F32, isOutput=True)
    h_d = nc.dram_tensor("h_scratch", [b, d], F32)

    with tile.TileContext(nc) as tc:
        with tc.tile_pool(name="persist", bufs=1) as pp:
            ident16 = pp.tile([P, P], F16, tag="ident16")
            _mk_identity(nc, ident16, 1.0)
            ones16 = pp.tile([1, P], F16, tag="ones16")
            nc.vector.memset(ones16, 1.0)
            db16 = pp.tile([1, 2, c], F16, tag="db16")
            nc.sync.dma_start(
                out=db16, in_=db16_d.rearrange("(o a) q -> o a q", o=1))

            # per-row threshold, one column per b-tile
            t_sb = pp.tile([P, nb], F32, tag="tsb")

            with tc.tile_pool(name="candp", bufs=1) as cp:
                cand = [cp.tile([P, (d // CHUNK) * 8], F32, tag=f"cand{bt}",
                                name=f"cand{bt}") for bt in range(nb)]
                with tc.tile_pool(name="xcpool", bufs=1) as xcp:
                    if ENCODE_MODE == "fp32r":
                        xcT = xcp.tile([P, nct, b], F32R, tag="xcT")
                    elif ENCODE_MODE == "fp8dr":
                        xcT = {
                            "h16": xcp.tile([P, nct, b], F16, tag="xh16",
                                            name="xh16"),
                            "x8h": xcp.tile([P, 2, nct // 2, b], F8,
                                            tag="x8h", name="x8h"),
                            "x8l": xcp.tile([P, 2, nct // 2, b], F8,
                                            tag="x8l", name="x8l"),
                        }
                    else:
                        xcT = xcp.tile([P, 2, nct, b], F16, tag="xcT")
                    _phase1_encode(nc, tc, ewT_d, eb16_d, h_d, xcT, xcT_d,
                                   cand, ones16, nb, nct, nslab, t_sb, k)

            out_acc = [pp.tile([P, c], F32, tag=f"oacc{bt}", name=f"oacc{bt}")
                       for bt in range(nb)]
            _phase3_decode(nc, tc, dwT_d, h_d, t_sb, db16, ones16, ident16,
                           out_acc, out_d, nb, nct, nslab, b, c)
    return nc


def _f16_split(a):
    hi = a.astype(np.float16)
    lo = (a - hi.astype(np.float32)).astype(np.float16)
    return np.stack([hi, lo])


def _il8(a):
    """[c, n] f32 -> [c/256, 128, 2, n] e4m3 DoubleRow interleave.

    Pairing: contraction index c = j*256 + i*128 + p maps to [j, p, i]."""
    c, n = a.shape
    v = a.reshape(c // 256, 2, P, n).transpose(0, 2, 1, 3)
    return np.ascontiguousarray(v).astype(ml_dtypes.float8_e4m3)


def run(x, encoder_w, encoder_b, decoder_w, decoder_b, k, trace=False):
    x = np.asarray(x, dtype=np.float32)
    encoder_w = np.asarray(encoder_w, dtype=np.float32)
    encoder_b = np.asarray(encoder_b, dtype=np.float32)
    decoder_w = np.asarray(decoder_w, dtype=np.float32)
    decoder_b = np.asarray(decoder_b, dtype=np.float32)
    k = int(k)
    b, r, c = x.shape
    d = encoder_w.shape[1]
    assert (b, r, c, d) == (B, R, C, D), (b, r, c, d)

    nc = build(k)
    if not nc.is_finalized():
        nc.finalize()
    in_maps = []
    for i in range(r):
        xc = x[:, i, :] - decoder_b[i][None, :]
        if ENCODE_MODE == "fp32r":
            in_maps.append({
                "xcT": np.ascontiguousarray(xc.T),
                "ewT": np.ascontiguousarray(encoder_w[i].T),
                "eb16": _f16_split(encoder_b[i]),
                "dwT16": np.ascontiguousarray(
                    decoder_w[i].T).astype(np.float16),
                "db16": _f16_split(decoder_b[i]),
            })
        elif ENCODE_MODE == "fp8dr":
            xcT = np.ascontiguousarray(xc.T)
            xh = xcT.astype(np.float16)
            xl = xcT - xh.astype(np.float32)
            ew64 = np.ascontiguousarray(encoder_w[i].T) * np.float32(WSCALE)
            wh = ew64.astype(np.float16)
            wl = ew64 - wh.astype(np.float32)
            in_maps.append({
                "xcTh": (xh.astype(np.float32)
                         * np.float32(F8SCALE)).astype(np.float16),
                "x8h": _il8(xh.astype(np.float32)),
                "x8l": _il8(xl * np.float32(F8SCALE)),
                "ewTh": wh,
                "w8h": _il8(wh.astype(np.float32)),
                "w8l": _il8(wl * np.float32(F8SCALE)),
                "eb16": _f16_split(encoder_b[i] * np.float32(WSCALE)),
                "dwT16": (np.ascontiguousarray(decoder_w[i].T)
                          * np.float32(1.0 / WSCALE)).astype(np.float16),
                "db16": _f16_split(decoder_b[i]),
            })
        else:
            # 64x-scaled hi/lo splits; decoder weights pre-scaled 1/64 so
            # the 64x-scaled masked h cancels in the decode matmul.
            in_maps.append({
                "xcT": _f16_split(np.ascontiguousarray(xc.T)),
                "ewT": _f16_split(
                    np.ascontiguousarray(encoder_w[i].T) * np.float32(WSCALE)),
                "eb16": _f16_split(encoder_b[i] * np.float32(WSCALE)),
                "dwT16": (np.ascontiguousarray(decoder_w[i].T)
                          * np.float32(1.0 / WSCALE)).astype(np.float16),
                "db16": _f16_split(decoder_b[i]),
            })
    res = run_bass_kernel_spmd(nc, in_maps, core_ids=list(range(N_CORES)),
                               trace=trace)
    out = np.empty((b, r, c), dtype=np.float32)
    for i in range(r):
        out[:, i, :] = res.results[i]["out"]
    return out, res


def kernel(x, encoder_w, encoder_b, decoder_w, decoder_b, k):
    out, _ = run(x, encoder_w, encoder_b, decoder_w, decoder_b, k)
    return out

